# revision 7
# baseline (speedup 1.0000x reference)
"""TRN2 Bass kernel for nn_AttentionCell (BitLinear GQA attention cell), v2.

Sharding (8 cores): data-parallel over batch (2) x tensor-parallel over the
4 KV head-groups (4 query heads each). Each core computes Q/K/V projections,
causal softmax attention for its 4 heads, and a row-parallel partial of the
output projection; the host sums the 4 partials per batch and applies the
final RMSNorm row scale.

v2 rework (cost-model driven, BIR-verifier clean):
 - projections / scores / stats run single-pass fp32r (1 cyc/row at
   free>=256): x, wq, wkv ship as float32r DRAM tensors; Q/K tiles are
   written as float32r by ACT/DVE evacuations (legal "rounding" producers)
 - S.T scores are ONE 65-deep fp32r matmul per 128-key block: rows 0-63 =
   K features, row 64 = ones x (-rowmax) aug; causal masking via shifted
   triangular -60000 f16 mask matmuls emitted first (start=True zeroes
   the psum bank)
 - exp evacuates PSUM bank-pairs in single [128,1024] ACT instructions,
   writing fp8e4m3 probabilities (exact row max keeps the top prob ~1)
 - PV runs DoubleRow fp8 at 0.5 cyc/row: V is split into fp8-hi + fp8-lo
   contraction planes (f16-level precision) padded to stride 80; the
   probs are broadcast (stride-0) into both planes; plane-0 carries a
   ones column producing the softmax denominators
 - Pool/GPSIMD cannot touch PSUM, so all PSUM evacuation goes ACT/DVE via
   a static load balancer; Pool handles SBUF-side work (masks, merges,
   squares); stats max-reduces are DVE
"""

import numpy as np
import ml_dtypes

import concourse.bass as bass
import concourse.bacc as bacc
import concourse.mybir as mybir
import concourse.tile as tile
from concourse.bass_utils import run_bass_kernel_spmd
from concourse.masks import make_identity

f32 = mybir.dt.float32
f32r = mybir.dt.float32r
f16 = mybir.dt.float16
bf16 = mybir.dt.bfloat16
f8 = mybir.dt.float8e4

EPS = np.float32(1.1920929e-07)
B, T, D = 2, 2048, 1024
H, HKV, HD = 16, 4, 64
NH = 4            # local (per-core) query heads
LF = NH * HD      # 256 local q features
P = 128
DT = D // P       # 8 d-tiles
CH = 4            # 512-wide token chunks
CW = 512
QB = T // P       # 16 query row blocks
KB = T // P       # 16 key blocks
MNEG = -60000.0
VP = 80           # fp8 V plane stride (16B-aligned for DoubleRow)

Exp = mybir.ActivationFunctionType.Exp
AOp = mybir.AluOpType
PM = mybir.MatmulPerfMode


def _build():
    nc = bacc.Bacc("TRN2", target_bir_lowering=False, debug=False)

    xn_d = nc.dram_tensor("xn", [D, T], f32r, kind="ExternalInput").ap()
    std_d = nc.dram_tensor("stdc", [P, KB], f32, kind="ExternalInput").ap()
    wq_d = nc.dram_tensor("wq", [D, LF], f32r, kind="ExternalInput").ap()
    wkv_d = nc.dram_tensor("wkv", [D, P], f32r, kind="ExternalInput").ap()
    wo_d = nc.dram_tensor("wo", [LF, D], f16, kind="ExternalInput").ap()
    yp = nc.dram_tensor("yp", [T, D], f32, kind="ExternalOutput").ap()
    ssqa = nc.dram_tensor("ssqa", [1, T], f32, kind="ExternalOutput").ap()

    # static ACT/DVE load balancer for PSUM evacuations (us estimates)
    load = {"DVE": 0.0, "ACT": 0.0}

    def evac(dst, src, w=0.66, prefer=None):
        eng = prefer
        if eng is None:
            eng = "ACT" if load["ACT"] <= load["DVE"] else "DVE"
        if eng == "ACT":
            nc.scalar.copy(dst, src)
            load["ACT"] += w * 1.25
        else:
            nc.vector.tensor_copy(dst, src)
            load["DVE"] += w

    with tile.TileContext(nc) as tc:
        with (
            tc.tile_pool(name="const", bufs=1) as const,
            tc.tile_pool(name="persist", bufs=1) as persist,
            tc.tile_pool(name="phm", bufs=12) as phm,
            tc.tile_pool(name="ptp", bufs=16) as ptp,
        ):
            ident32 = const.tile([P, P], f32, tag="ident32")
            make_identity(nc, ident32[:])
            identh = const.tile([P, P], f16, tag="identh")
            nc.vector.tensor_copy(identh[:], ident32[:])
            # stats mask (S layout [query-part, key-free]): key > query -> MNEG
            mask_s = const.tile([P, P], f16, tag="mask_s")
            nc.gpsimd.memset(mask_s[:], 0.0)
            nc.gpsimd.affine_select(
                out=mask_s[:], in_=mask_s[:],
                compare_op=AOp.is_ge, fill=MNEG,
                base=0, pattern=[[-1, P]], channel_multiplier=1,
            )
            # S.T masks [key-part k, query-free j], shifted variants:
            # fill where j < s + k (query above/before the diagonal)
            maskst = []
            for si in range(4):
                m = const.tile([P, CW], f16, tag=f"maskst{si}",
                               name=f"maskst{si}")
                nc.gpsimd.memset(m[:], 0.0)
                nc.gpsimd.affine_select(
                    out=m[:], in_=m[:],
                    compare_op=AOp.is_ge, fill=MNEG,
                    base=-si * P, pattern=[[1, CW]], channel_multiplier=-1,
                )
                maskst.append(m)
            onesT = const.tile([1, T], f32, tag="onesT")
            nc.gpsimd.memset(onesT[:], 1.0)

            xsb = persist.tile([P, DT, T], f32r, tag="xsb")
            xr = xn_d.rearrange("(dt p) t -> p dt t", p=P)
            wkv_sb = persist.tile([P, DT, P], f32r, tag="wkv_sb")
            nc.sync.dma_start(
                out=wkv_sb[:], in_=wkv_d.rearrange("(dt p) f -> p dt f", p=P)
            )
            # chunk-0 x on the cheap SP queue, interleaved with the weights,
            # so the first projection matmuls start ~2.5us in
            for d in range(4):
                nc.sync.dma_start(out=xsb[:, d:d + 1, 0:CW],
                                  in_=xr[:, d:d + 1, 0:CW])
            wq_sb = persist.tile([P, DT, LF], f32r, tag="wq_sb")
            nc.sync.dma_start(
                out=wq_sb[:], in_=wq_d.rearrange("(dt p) f -> p dt f", p=P)
            )
            for d in range(4, DT):
                nc.sync.dma_start(out=xsb[:, d:d + 1, 0:CW],
                                  in_=xr[:, d:d + 1, 0:CW])
            stdc = persist.tile([P, KB], f32, tag="stdc")
            nc.sync.dma_start(out=stdc[:], in_=std_d[:])

            # Q/K feature tiles (f32r): rows 0-63 features, row 64 aug
            QTall = [persist.tile([65, T], f32r, tag=f"qt{h}", name=f"qt{h}")
                     for h in range(NH)]
            KTall = persist.tile([65, T], f32r, tag="kt")
            nc.vector.tensor_copy(KTall[64:65, :], onesT[:])
            # V in fp8 hi/lo planes + ones/zeros denominator column
            Vsb = [persist.tile([P, 2, VP], f8, tag=f"v{kb}",
                                name=f"v{kb}") for kb in range(KB)]
            for kb in range(KB):
                nc.gpsimd.memset(Vsb[kb][:, 0, HD:HD + 1], 1.0)
                nc.gpsimd.memset(Vsb[kb][:, 1, HD:HD + 1], 0.0)
            ones1h = persist.tile([1, P], f16, tag="ones1h")
            nc.gpsimd.memset(ones1h[:], 1.0)
            onesc = persist.tile([P, 1], f16, tag="onesc")
            nc.gpsimd.memset(onesc[:], 1.0)
            aTh = persist.tile([P, 2, T], f16, tag="aTh")
            ssqrow = persist.tile([1, T], f32, tag="ssqrow")

            # chunks 1-3 on the Pool SWDGE queues (after the const setup)
            for c in range(1, CH):
                cs = slice(c * CW, (c + 1) * CW)
                for dh in range(4):
                    ds = slice(2 * dh, 2 * dh + 2)
                    nc.gpsimd.dma_start(out=xsb[:, ds, cs],
                                        in_=xr[:, ds, cs])
            wo_sb = persist.tile([P, 2, D], f16, tag="wo_sb")
            nc.sync.dma_start(
                out=wo_sb[:], in_=wo_d.rearrange("(ft p) o -> p ft o", p=P)
            )

            with tc.tile_pool(name="ps_stat", bufs=2,
                              space="PSUM") as ps_stat:
                mall = {}
                queue = []       # gated stats/nm work queue: (rc, closure)
                pnm_alloc = {}

                def emit_stat_unit(h, qb):
                    """Row-max for query block qb, head h: fp32r S-layout
                    matmuls + causal diag mask + DVE max reduces; partial
                    maxes merged on Pool (SBUF-only)."""
                    rc, j = divmod(qb, 4)
                    if j == 0:
                        mall[(h, rc)] = phm.tile([P, 4], f32, tag="mall",
                                                 name=f"mall_{h}_{rc}")
                    ml = mall[(h, rc)]
                    qs = slice(qb * P, (qb + 1) * P)
                    nk = (qb + 1) * P
                    nch = (nk + CW - 1) // CW
                    ts = []
                    for ic in range(nch):
                        lo = ic * CW
                        w = min(CW, nk - lo)
                        last = ic == nch - 1
                        pS = ps_stat.tile([P, CW], f32, tag="pstat",
                                          name=f"pstat_{h}_{qb}_{ic}")
                        nc.tensor.matmul(
                            pS[:, 0:w],
                            QTall[h][0:HD, qs],
                            KTall[0:HD, lo:lo + w],
                            start=True, stop=not last,
                        )
                        if last:
                            nc.tensor.matmul(pS[:, w - P:w], identh[:],
                                             mask_s[:], start=False,
                                             stop=True)
                        dst = ml[:, j:j + 1] if nch == 1 else None
                        t = phm.tile([P, 1], f32, tag="tcol",
                                     name=f"t_{h}_{qb}_{ic}")
                        nc.vector.tensor_reduce(
                            out=dst if dst is not None else t[:],
                            in_=pS[:, 0:w],
                            axis=mybir.AxisListType.X, op=AOp.max)
                        load["DVE"] += 0.28 + w * 0.00104
                        if dst is None:
                            ts.append(t)
                    # merge partial maxes on Pool (SBUF only)
                    while len(ts) > 2:
                        nc.vector.tensor_max(ts[0][:], ts[0][:], ts[1][:])
                        ts.pop(1)
                    if len(ts) == 2:
                        nc.vector.tensor_max(ml[:, j:j + 1], ts[0][:],
                                             ts[1][:])
                    elif len(ts) == 1:
                        nc.vector.tensor_copy(ml[:, j:j + 1], ts[0][:])

                def emit_nm(h, rc):
                    # -max into the QTall aug row via transpose + DMA reshape
                    rs = slice(rc * CW, (rc + 1) * CW)
                    pnm = pnm_alloc["fn"](h, rc)
                    nc.tensor.transpose(pnm[:], mall[(h, rc)][:], ident32[:])
                    nm4 = phm.tile([4, P], f32r, tag="nm4",
                                   name=f"nm4_{h}_{rc}")
                    nc.scalar.mul(nm4[:], pnm[:], -1.0)
                    load["ACT"] += 0.2
                    nc.sync.dma_start(out=QTall[h][64:65, rs], in_=nm4[:])

                for rc in range(4):
                    for h in range(NH):
                        for j in range(4):
                            queue.append((rc, lambda hh=h, qq=rc * 4 + j:
                                          emit_stat_unit(hh, qq)))
                        queue.append((rc, lambda hh=h, rr=rc:
                                      emit_nm(hh, rr)))

                fill_ctr = [0]

                def filler(max_rc, budget=1, stride=1):
                    fill_ctr[0] += 1
                    if stride > 1 and fill_ctr[0] % stride != 0:
                        return
                    while budget > 0 and queue and queue[0][0] <= max_rc:
                        queue.pop(0)[1]()
                        budget -= 1

                def drain_for(rc):
                    idxs = [i for i, e in enumerate(queue) if e[0] == rc]
                    if idxs:
                        for _ in range(idxs[-1] + 1):
                            queue.pop(0)[1]()

                # ---------------- phase 2: projections ----------------
                with (
                    tc.tile_pool(name="ph2v", bufs=2) as ph2v,
                    tc.tile_pool(name="ps_q", bufs=3, space="PSUM") as ps_q,
                    tc.tile_pool(name="ps_kv", bufs=1, space="PSUM") as ps_kv,
                    tc.tile_pool(name="ps_vt", bufs=2, space="PSUM") as ps_vt,
                ):
                    pnm_alloc["fn"] = lambda hh, rr: ps_vt.tile(
                        [4, P], f32, tag="pvt", name=f"pnm_{hh}_{rr}")
                    for c in range(CH):
                        cs = slice(c * CW, (c + 1) * CW)
                        psQ = [ps_q.tile([P, CW], f32, tag="psq",
                                         name=f"psq_{c}_{i}")
                               for i in range(2)]
                        psKV = ps_kv.tile([P, CW], f32, tag="pskv")
                        for d in range(DT):
                            xt = xsb[:, d, cs]
                            first, last = d == 0, d == DT - 1
                            nc.tensor.matmul(psKV[:], wkv_sb[:, d, :],
                                             xt, start=first, stop=last)
                            for ft in range(2):
                                wslc = wq_sb[:, d, ft * P:(ft + 1) * P]
                                nc.tensor.matmul(psQ[ft][:], wslc, xt,
                                                 start=first, stop=last)
                            filler(c - 1, budget=1)
                        # evacuate Q/K features as f32r (ACT/DVE balanced)
                        for ft in range(2):
                            for sub in range(2):
                                h = 2 * ft + sub
                                evac(QTall[h][0:HD, cs],
                                     psQ[ft][sub * HD:(sub + 1) * HD, :])
                        evac(KTall[0:HD, cs], psKV[0:HD, :])
                        vt = ph2v.tile([HD, CW], f32, tag="vt")
                        evac(vt[:], psKV[HD:P, :])
                        for s4 in range(4):
                            kb = c * 4 + s4
                            pvt = ps_vt.tile([P, HD], f32, tag="pvt")
                            nc.tensor.transpose(
                                pvt[:], vt[:, s4 * P:(s4 + 1) * P],
                                ident32[0:HD, 0:HD])
                            # V8hi = fp8(V*std); vts = f16(V*std);
                            # V8lo = fp8(vts - V8hi)
                            vts = ph2v.tile([P, HD], f16, tag="vts")
                            nc.scalar.mul(Vsb[kb][:, 0, 0:HD], pvt[:],
                                          stdc[:, kb:kb + 1])
                            nc.scalar.mul(vts[:], pvt[:],
                                          stdc[:, kb:kb + 1])
                            load["ACT"] += 0.4
                            nc.vector.tensor_sub(Vsb[kb][:, 1, 0:HD],
                                                 vts[:],
                                                 Vsb[kb][:, 0, 0:HD])
                            load["DVE"] += 0.13
                        filler(c - 1, budget=2)

                # ------- phases 3-6: attention + O-proj, per row-chunk -------
                with (
                    tc.tile_pool(name="ph3", bufs=6) as ph3,
                    tc.tile_pool(name="ph6", bufs=5) as ph6,
                    tc.tile_pool(name="ps_st", bufs=2, space="PSUM") as ps_st,
                    tc.tile_pool(name="ps_y", bufs=2, space="PSUM") as ps_y,
                ):
                    pnm_alloc["fn"] = lambda hh, rr: ps_y.tile(
                        [4, P], f32, tag="psy", name=f"pnm_{hh}_{rr}")

                    def emit_attn(h, rc):
                        rs = slice(rc * CW, (rc + 1) * CW)
                        ft, sub = h // 2, h % 2
                        npair = 2 * (rc + 1)
                        psO = ps_y.tile([65, CW], f32, tag="psy",
                                        name=f"pso_{h}_{rc}")
                        for pr in range(npair):
                            pst2 = ps_st.tile([P, 2 * CW], f32, tag="pst",
                                              name=f"pst_{h}_{rc}_{pr}")
                            pt8 = ptp.tile([P, 2, CW], f8, tag="pt8",
                                           name=f"pt8_{h}_{rc}_{pr}")
                            for half in range(2):
                                kc = 2 * pr + half
                                kslc = slice(kc * P, (kc + 1) * P)
                                off = half * CW
                                if kc >= rc * 4:
                                    s = kc * P - rc * CW
                                    # S.T first (start=True pending-zeroes
                                    # the bank); then the all-neg left strip
                                    # (pending region) and the diag triangle
                                    # (written region) as separate mask
                                    # matmuls so each region is uniform
                                    nc.tensor.matmul(
                                        pst2[:, off + s:off + CW],
                                        KTall[:, kslc],
                                        QTall[h][:, rc * CW + s:
                                                 (rc + 1) * CW],
                                        start=True, stop=False)
                                    if s > 0:
                                        nc.tensor.matmul(
                                            pst2[:, off:off + s], identh[:],
                                            maskst[s // P][:, 0:s],
                                            start=False, stop=False)
                                    nc.tensor.matmul(
                                        pst2[:, off + s:off + s + P],
                                        identh[:],
                                        maskst[s // P][:, s:s + P],
                                        start=False, stop=True)
                                else:
                                    nc.tensor.matmul(
                                        pst2[:, off:off + CW],
                                        KTall[:, kslc],
                                        QTall[h][:, rs],
                                        start=True, stop=True)
                            nc.scalar.activation(
                                pt8[:].rearrange("p a b -> p (a b)"),
                                pst2[:], Exp)
                            load["ACT"] += 1.04
                            for half in range(2):
                                kc = 2 * pr + half
                                lo = max(kc * P - rc * CW, 0)
                                w = CW - lo
                                rhs = pt8[:, half:half + 1,
                                          lo:CW].broadcast_to((P, 2, w))
                                nc.tensor.matmul(
                                    psO[:, lo:CW], Vsb[kc][:, :, 0:HD + 1],
                                    rhs, start=(pr == 0 and half == 0),
                                    stop=(pr == npair - 1 and half == 1),
                                    perf_mode=PM.DoubleRow)
                            filler(rc + 1, stride=2)
                        # 1/denom (f16) -> broadcast via rank-1 matmul
                        rd16 = ph3.tile([1, CW], f16, tag="rd16")
                        with nc.allow_low_precision(reason="1/denom f16"):
                            nc.vector.reciprocal(rd16[:], psO[64:65, :])
                        load["DVE"] += 0.66
                        dbc = ps_y.tile([HD, CW], f32, tag="psy",
                                        name=f"dbc_{rc}_{h}")
                        nc.tensor.matmul(dbc[:], ones1h[0:1, 0:HD], rd16[:],
                                         start=True, stop=True)
                        # only one PSUM input allowed per vector op: evacuate
                        # the PV rows to SBUF, then scale by the broadcast
                        au = ph3.tile([HD, CW], f32, tag="au")
                        evac(au[:], psO[0:HD, :],
                             prefer="DVE" if rc >= 3 else None)
                        nc.vector.tensor_mul(
                            aTh[sub * HD:(sub + 1) * HD, ft, rs],
                            au[:], dbc[:])
                        load["DVE"] += 0.66
                        filler(rc + 1, stride=2)

                    def emit_ssq_oproj(rc):
                        rs = slice(rc * CW, (rc + 1) * CW)
                        # ssq of normalized attn rows (partial, this core)
                        psq = ps_y.tile([1, CW], f32, tag="psy",
                                        name=f"psq_{rc}")
                        for ft in range(2):
                            sqt = ph6.tile([P, CW], f16, tag="sqt")
                            nc.gpsimd.tensor_mul(sqt[:], aTh[:, ft, rs],
                                                 aTh[:, ft, rs])
                            nc.tensor.matmul(psq[:], onesc[:], sqt[:],
                                             start=(ft == 0), stop=(ft == 1))
                        evac(ssqrow[0:1, rs], psq[:])

                        # O-projection for this row chunk (f16)
                        for j in range(4):
                            qb = rc * 4 + j
                            qs = slice(qb * P, (qb + 1) * P)
                            for oc in range(2):
                                os_ = slice(oc * CW, (oc + 1) * CW)
                                psY = ps_y.tile([P, CW], f32, tag="psy",
                                                name=f"psY_{qb}_{oc}")
                                for ft in range(2):
                                    nc.tensor.matmul(psY[:],
                                                     aTh[:, ft, qs],
                                                     wo_sb[:, ft, os_],
                                                     start=(ft == 0),
                                                     stop=(ft == 1))
                                ysb = ph6.tile([P, CW], f32, tag="ysb")
                                evac(ysb[:], psY[:])
                                nc.sync.dma_start(out=yp[qs, os_], in_=ysb[:])
                            filler(rc + 1, stride=2)

                    for rc in range(4):
                        drain_for(rc)   # must-have stats for this rc
                        for h in range(NH):
                            emit_attn(h, rc)
                        emit_ssq_oproj(rc)
                    nc.sync.dma_start(out=ssqa[:], in_=ssqrow[:])
    nc.finalize()
    return nc


def _ternary(w):
    th = np.abs(w).mean(dtype=np.float64)
    return (np.sign(w) * (np.abs(w) > th)).astype(np.float32)


_CACHE = {}


def kernel(x, q_w, q_g, k_w, k_g, v_w, o_w, o_g, qk_gain):
    x = np.asarray(x, np.float32)
    wq_eff = (_ternary(np.asarray(q_w)) * np.asarray(q_g)[None, :]
              * np.float32(qk_gain) / np.float32(np.sqrt(np.float32(HD))))
    wk_eff = _ternary(np.asarray(k_w)) * np.asarray(k_g)[None, :]
    wo_eff = _ternary(np.asarray(o_w)) * np.asarray(o_g)[None, :]
    wqT = np.ascontiguousarray(wq_eff.T)                         # [D, H*HD]
    wkT = np.ascontiguousarray(wk_eff.T)                         # [D, HKV*HD]
    wvT = np.ascontiguousarray(np.asarray(v_w, np.float32).T)    # [D, HKV*HD]
    woT = np.ascontiguousarray(wo_eff.T).astype(np.float16)      # [D, D]

    # per-token rms scales (host); x_hat = x * r, V un-normalized on device
    xs = x.astype(np.float64)
    ssq = (xs * xs).mean(-1) + np.float64(EPS)
    r = (1.0 / np.sqrt(ssq)).astype(np.float32)                  # [B, T]
    std = np.sqrt(ssq).astype(np.float32)                        # [B, T]
    xn = (x * r[:, :, None]).astype(np.float32)

    if "nc" not in _CACHE:
        _CACHE["nc"] = _build()
    nc = _CACHE["nc"]

    in_maps = []
    for core in range(8):
        b, g = divmod(core, 4)
        xnT = np.ascontiguousarray(xn[b].T)                      # [D, T] f32
        wkv_c = np.concatenate(
            [wkT[:, g * HD:(g + 1) * HD], wvT[:, g * HD:(g + 1) * HD]],
            axis=1)
        in_maps.append({
            "xn": xnT,
            "stdc": np.ascontiguousarray(std[b].reshape(KB, P).T),
            "wq": np.ascontiguousarray(wqT[:, g * LF:(g + 1) * LF]),
            "wkv": np.ascontiguousarray(wkv_c),
            "wo": np.ascontiguousarray(woT[g * LF:(g + 1) * LF, :]),
        })
    _CACHE["in_maps"] = in_maps
    res = run_bass_kernel_spmd(nc, in_maps, list(range(8)))

    out = np.empty((B, T, D), np.float32)
    for b in range(B):
        ssq_a = np.zeros((T,), np.float32)
        ysum = np.zeros((T, D), np.float32)
        for g in range(4):
            rr = res.results[b * 4 + g]
            ysum += rr["yp"]
            ssq_a += rr["ssqa"][0]
        ro = 1.0 / np.sqrt(ssq_a / np.float32(D) + EPS)
        out[b] = ysum * ro[:, None]
    return out


if __name__ == "__main__":
    data = np.load("/root/problem/inputs.npz")
    out = kernel(**{k: data[k] for k in data.files})
    ref = np.load("/root/problem/ref_out.npy")
    d = out.astype(np.float64) - ref.astype(np.float64)
    rv = (d * d).sum() / (ref.astype(np.float64) ** 2).sum()
    print("resid_var=%.3e relerr=%.3e absmax=%.3g" %
          (rv, np.sqrt(rv), np.abs(d).max()))
